# revision 50
# baseline (speedup 1.0000x reference)
"""Trainium2 Bass kernel for CustomHyperSemanticMessagePassing.

Hypergraph multi-head attention message passing, N=4096 nodes, E=4096 edges,
DEG=CARD=8, D=256, H=8 heads. Sharding: data-parallel over nodes (512/core).

Final design (engine-balanced, bf16 tables, HW-validated gather form):
  - K|V table [N, 512] bf16 (K half (h,e) layout with bk folded; V half in
    (e,h) layout), EK table [E, 256] bf16, built on PE + converted via
    DVE/Act, stored to DRAM-pool tiles (dep-tracked vs the gathers).
  - Gathers use ONLY the single-index-per-partition indirect DMA form
    (multi-index offset APs are mis-ordered by real HW descriptor gen).
  - Self-pair trick: host reorders each round's pair set so slot 0 is the
    node itself; only 7 rows gathered per round, own row fetched once/tile
    and its q.K score reused across rounds.
  - Engine balance: Pool = gathers + small tree levels (s4,s5,sadd,c2);
    DVE = score/value multiplies (bf16 2x mode) + big tree levels +
    combines; Act = exp, phase-T V/EK conversion copies, PSUM copies,
    relu; PE = table matmuls, transposes, out-proj.
  - bv is folded into the output bias host-side (sum of softmax weights
    is 1 per head), so the V table needs no bias add.
  Softmax without max-subtraction (scores are O(1) bounded); per-round
  partial exp sums / weighted sums combine by plain addition.

"""
import numpy as np

import bass_rust
import orjson
import concourse.bass as bass
import concourse.tile as tile
import concourse.bass_utils as bass_utils
import concourse.bass2jax as bass2jax
from concourse import mybir
from concourse.masks import make_identity

F32 = mybir.dt.float32
BF16 = mybir.dt.bfloat16
FP8 = mybir.dt.float8e4
I32 = mybir.dt.int32

N, E, D, EDGE_DIM = 4096, 4096, 256, 64
H, DH, DEG, CARD = 8, 32, 8, 8
L = DEG * CARD
NCORES = 8
NSH = N // NCORES          # nodes per core
NT = NSH // 128            # 128-node tiles per core
NM = N // 128              # table tiles


# ---------------------------------------------------------------------------
# walrus workaround: this build accepts only one sync-wait per instruction;
# split extras into injected single-wait NoOps at the BIR-JSON level.
_ORIG_COMPILE = bass_utils.compile_bir_kernel
_ctr = [0]


def _split_multiwaits(bir_json: bytes) -> bytes:
    bir = orjson.loads(bir_json)
    changed = False
    for f in bir.get("functions", []):
        for blk in f.get("blocks", []):
            out = []
            for ins in blk.get("instructions", []):
                si = ins.get("sync_info")
                waits = (si or {}).get("on_wait") or []
                if len(waits) > 1 and ins.get("engine") not in (None, "Unassigned"):
                    changed = True
                    for w in waits[:-1]:
                        _ctr[0] += 1
                        out.append({
                            "debug": ins.get("debug"),
                            "engine": ins["engine"],
                            "ins": [], "outs": [],
                            "name": f"WSPLIT-{_ctr[0]}",
                            "opcode": "NoOp",
                            "sync_info": {"on_wait": [w], "on_update": []},
                        })
                    si["on_wait"] = waits[-1:]
                out.append(ins)
            blk["instructions"] = out
    return orjson.dumps(bir) if changed else bir_json


def _patched_compile(bir_json, tmpdir, neff_name="file.neff"):
    return _ORIG_COMPILE(_split_multiwaits(bytes(bir_json)), tmpdir,
                         neff_name=neff_name)


def _install_patch():
    bass_utils.compile_bir_kernel = _patched_compile
    bass2jax.compile_bir_kernel = _patched_compile


_install_patch()


# ---------------------------------------------------------------------------
def build_nc():
    nc = bass.Bass(num_devices=NCORES)
    # replicated inputs
    xT = nc.declare_dram_parameter("xT", [D, N], BF16, isOutput=False)
    eaT = nc.declare_dram_parameter("eaT", [EDGE_DIM, E], BF16, isOutput=False)
    wkc = nc.declare_dram_parameter("wkc", [D, D], BF16, isOutput=False)
    wvc = nc.declare_dram_parameter("wvc", [D, D], BF16, isOutput=False)
    wqc = nc.declare_dram_parameter("wqc", [D, D], BF16, isOutput=False)
    wek = nc.declare_dram_parameter("wek", [EDGE_DIM, D], BF16, isOutput=False)
    owT = nc.declare_dram_parameter("owT", [D, D], BF16, isOutput=False)
    bkv_b = nc.declare_dram_parameter("bkv_b", [128, 2 * D], F32, isOutput=False)
    bq_b = nc.declare_dram_parameter("bq_b", [128, D], F32, isOutput=False)
    bo_b = nc.declare_dram_parameter("bo_b", [128, D], F32, isOutput=False)
    # per-core inputs
    xq_own = nc.declare_dram_parameter("xq_own", [D, NSH], BF16, isOutput=False)
    pu = nc.declare_dram_parameter("pu", [NSH, L], I32, isOutput=False)
    pe = nc.declare_dram_parameter("pe", [NSH, DEG], I32, isOutput=False)
    self_idx = nc.declare_dram_parameter("self_idx", [128, NT], I32, isOutput=False)
    # output
    out = nc.declare_dram_parameter("out", [NSH, D], F32, isOutput=True)

    AX = mybir.AxisListType.X
    ADD = mybir.AluOpType.add
    MUL = mybir.AluOpType.mult

    with tile.TileContext(nc) as tc, \
         tc.tile_pool(name="wpool", bufs=1) as wp, \
         tc.tile_pool(name="stage", bufs=2) as stg, \
         tc.tile_pool(name="qpool", bufs=NT + 1) as qp, \
         tc.tile_pool(name="gpool", bufs=4) as gp, \
         tc.tile_pool(name="spool", bufs=4) as sp_, \
         tc.tile_pool(name="tpool", bufs=2) as tp, \
         tc.tile_pool(name="psA", bufs=2, space="PSUM") as psA, \
         tc.tile_pool(name="psB", bufs=2, space="PSUM") as psB, \
         tc.tile_pool(name="psC", bufs=2, space="PSUM") as psC, \
         tc.tile_pool(name="dpool", bufs=1, space="DRAM") as dpl:

        kv_tab = dpl.tile([N, 2 * D], BF16)
        ek_tab = dpl.tile([E, D], BF16)

        # ---- resident weights / biases / x / ea ----
        wk_t = wp.tile([128, 2, D], BF16)
        nc.sync.dma_start(out=wk_t[:], in_=wkc[:].rearrange("(c k) o -> k c o", c=2))
        wv_t = wp.tile([128, 2, D], BF16)
        nc.sync.dma_start(out=wv_t[:], in_=wvc[:].rearrange("(c k) o -> k c o", c=2))
        wq_t = wp.tile([128, 2, D], BF16)
        nc.sync.dma_start(out=wq_t[:], in_=wqc[:].rearrange("(c k) o -> k c o", c=2))
        wek_t = wp.tile([EDGE_DIM, D], BF16)
        nc.sync.dma_start(out=wek_t[:], in_=wek[:])
        owT_t = wp.tile([128, 2, D], BF16)
        nc.sync.dma_start(out=owT_t[:], in_=owT[:].rearrange("(c k) o -> k c o", c=2))
        bkv_t = wp.tile([128, 2 * D], F32)
        nc.scalar.dma_start(out=bkv_t[:], in_=bkv_b[:])
        bq_t = wp.tile([128, D], F32)
        nc.scalar.dma_start(out=bq_t[:], in_=bq_b[:])
        bo_t = wp.tile([128, D], F32)
        nc.scalar.dma_start(out=bo_t[:], in_=bo_b[:])
        ident = wp.tile([128, 128], F32)
        make_identity(nc, ident[:])

        # x resident: [k=128, chalf=2, m=32, n=128]; split load by chalf
        xt = wp.tile([128, 2, NM, 128], BF16)
        for ci, eng in ((0, nc.sync), (1, nc.scalar)):
            eng.dma_start(
                out=xt[:, ci, :, :],
                in_=xT[bass.ts(ci, 128), :].rearrange("k (m n) -> k m n", m=NM))
        # ea resident: [64, m=32, n=128]
        eat = wp.tile([EDGE_DIM, NM, 128], BF16)
        nc.sync.dma_start(
            out=eat[:], in_=eaT[:].rearrange("e (m n) -> e m n", m=NM))
        # own q source
        xqo = wp.tile([128, 2, NT, 128], BF16)
        nc.scalar.dma_start(
            out=xqo[:],
            in_=xq_own[:].rearrange("(c k) (t n) -> k c t n", c=2, t=NT))
        # indices
        pu_t = wp.tile([128, NT, L], I32)
        nc.sync.dma_start(
            out=pu_t[:], in_=pu[:].rearrange("(t p) l -> p t l", t=NT))
        pe_t = wp.tile([128, NT, DEG], I32)
        nc.sync.dma_start(
            out=pe_t[:], in_=pe[:].rearrange("(t p) l -> p t l", t=NT))
        sidx_t = wp.tile([128, NT], I32)
        nc.sync.dma_start(out=sidx_t[:], in_=self_idx[:])

        # ---- phase T: KV table ----
        for mg in range(NM // 4):

            k_st = stg.tile([128, 4, D], BF16, tag="kst")
            v_st = stg.tile([128, 4, D], BF16, tag="vst")
            for mi in range(0, 4, 2):
                pk2 = psA.tile([128, 2 * D], F32, space="PSUM", tag="pk2")
                pv2 = psA.tile([128, 2 * D], F32, space="PSUM", tag="pv2")
                for j in range(2):
                    m = mg * 4 + mi + j
                    nc.tensor.matmul(out=pk2[:, j * D:(j + 1) * D],
                                     lhsT=xt[:, 0, m, :], rhs=wk_t[:, 0, :],
                                     start=True, stop=False)
                    nc.tensor.matmul(out=pk2[:, j * D:(j + 1) * D],
                                     lhsT=xt[:, 1, m, :], rhs=wk_t[:, 1, :],
                                     start=False, stop=True)
                    nc.tensor.matmul(out=pv2[:, j * D:(j + 1) * D],
                                     lhsT=xt[:, 0, m, :], rhs=wv_t[:, 0, :],
                                     start=True, stop=False)
                    nc.tensor.matmul(out=pv2[:, j * D:(j + 1) * D],
                                     lhsT=xt[:, 1, m, :], rhs=wv_t[:, 1, :],
                                     start=False, stop=True)
                nc.vector.tensor_tensor(
                    out=k_st[:, mi:mi + 2, :],
                    in0=pk2[:].rearrange("p (j d) -> p j d", j=2),
                    in1=bkv_t[:, 0:D].unsqueeze(1).to_broadcast([128, 2, D]),
                    op=ADD)
                if mi % 4 == 0:
                    nc.scalar.copy(
                        out=v_st[:, mi:mi + 2, :],
                        in_=pv2[:].rearrange("p (j d) -> p j d", j=2))
                else:
                    nc.vector.tensor_scalar(
                        out=v_st[:, mi:mi + 2, :],
                        in0=pv2[:].rearrange("p (j d) -> p j d", j=2),
                        scalar1=0.0, scalar2=None, op0=ADD)

            kv_rows = kv_tab[bass.ts(mg, 512), :].rearrange(
                "(s p) d -> p s d", s=4)
            eng = (nc.sync, nc.scalar)[mg % 2]
            eng.dma_start(out=kv_rows[:, :, 0:D], in_=k_st[:])
            eng = (nc.scalar, nc.sync)[mg % 2]
            eng.dma_start(out=kv_rows[:, :, D:2 * D], in_=v_st[:])

        # ---- phase T: EK table ----
        for mg in range(NM // 4):
            ek_st = stg.tile([128, 4, D], BF16, tag="ekst")
            for mi in range(0, 4, 2):
                pe2 = psA.tile([128, 2 * D], F32, space="PSUM", tag="pk2")
                for j in range(2):
                    m = mg * 4 + mi + j
                    nc.tensor.matmul(out=pe2[:, j * D:(j + 1) * D],
                                     lhsT=eat[:, m, :], rhs=wek_t[:],
                                     start=True, stop=True)
                nc.scalar.copy(
                    out=ek_st[:, mi:mi + 2, :],
                    in_=pe2[:].rearrange("p (j d) -> p j d", j=2))
            eng = (nc.sync, nc.scalar)[mg % 2]
            eng.dma_start(
                out=ek_tab[bass.ts(mg, 512), :].rearrange("(s p) d -> p s d", s=4),
                in_=ek_st[:])

        # ---- phase T: q for own nodes ----
        q_tiles = []
        for t in range(NT):
            pq = psB.tile([128, D], F32, space="PSUM", tag="p256")
            nc.tensor.matmul(out=pq[:], lhsT=xqo[:, 0, t, :], rhs=wq_t[:, 0, :],
                             start=True, stop=False)
            nc.tensor.matmul(out=pq[:], lhsT=xqo[:, 1, t, :], rhs=wq_t[:, 1, :],
                             start=False, stop=True)
            q_t = qp.tile([128, D], BF16, tag=f"q{t}")
            nc.vector.tensor_tensor(out=q_t[:], in0=pq[:], in1=bq_t[:], op=ADD)
            q_tiles.append(q_t)

        # ---- phase A ----
        for t in range(NT):
            q_t = q_tiles[t]
            # EK gathers (single-index per partition; HW-validated form)
            ekg = gp.tile([128, DEG, D], BF16, tag="ekg")
            for dg in range(DEG):
                nc.gpsimd.indirect_dma_start(
                    out=ekg[:, dg, :], out_offset=None, in_=ek_tab[:],
                    in_offset=bass.IndirectOffsetOnAxis(
                        ap=pe_t[:, t, dg:dg + 1], axis=0))
            # prode[p, d, (h e)] = EK * q  (Pool reads fp8)
            prode = tp.tile([128, DEG, D], BF16, tag="prode")
            nc.vector.tensor_tensor(
                out=prode[:], in0=ekg[:],
                in1=q_t[:].unsqueeze(1).to_broadcast([128, DEG, D]), op=MUL)
            # qek tree over e: [128, d, h, 32] -> [128, d, h]
            pv = prode[:].rearrange("p d (h e) -> p d h e", h=H)
            qk1 = tp.tile([128, DEG, H, 16], BF16, tag="qk1")
            nc.vector.tensor_tensor(out=qk1[:], in0=pv[:, :, :, 0:16],
                                    in1=pv[:, :, :, 16:32], op=ADD)
            qk2 = tp.tile([128, DEG, H, 8], BF16, tag="qk2")
            nc.vector.tensor_tensor(out=qk2[:], in0=qk1[:, :, :, 0:8],
                                    in1=qk1[:, :, :, 8:16], op=ADD)
            qk3 = tp.tile([128, DEG, H, 4], BF16, tag="qk3")
            nc.vector.tensor_tensor(out=qk3[:], in0=qk2[:, :, :, 0:4],
                                    in1=qk2[:, :, :, 4:8], op=ADD)
            qk4 = tp.tile([128, DEG, H, 2], BF16, tag="qk4")
            nc.vector.tensor_tensor(out=qk4[:], in0=qk3[:, :, :, 0:2],
                                    in1=qk3[:, :, :, 2:4], op=ADD)
            qek = tp.tile([128, DEG, H], BF16, tag="qek")
            nc.vector.tensor_tensor(out=qek[:], in0=qk4[:, :, :, 0],
                                    in1=qk4[:, :, :, 1], op=ADD)

            # own K|V row (self pair lives in slot 0 of every round)
            skv = tp.tile([128, 2 * D], BF16, tag="skv")
            nc.gpsimd.indirect_dma_start(
                out=skv[:], out_offset=None, in_=kv_tab[:],
                in_offset=bass.IndirectOffsetOnAxis(
                    ap=sidx_t[:, t:t + 1], axis=0))
            pks = tp.tile([128, D], BF16, tag="pks")
            nc.vector.tensor_tensor(out=pks[:], in0=skv[:, 0:D], in1=q_t[:],
                                    op=MUL)
            pk4 = pks[:].rearrange("p (h e) -> p h e", h=H)
            t1 = tp.tile([128, H, 16], BF16, tag="t1")
            nc.vector.tensor_tensor(out=t1[:], in0=pk4[:, :, 0:16],
                                    in1=pk4[:, :, 16:32], op=ADD)
            t2 = tp.tile([128, H, 8], BF16, tag="t2")
            nc.vector.tensor_tensor(out=t2[:], in0=t1[:, :, 0:8],
                                    in1=t1[:, :, 8:16], op=ADD)
            t3 = tp.tile([128, H, 4], BF16, tag="t3")
            nc.vector.tensor_tensor(out=t3[:], in0=t2[:, :, 0:4],
                                    in1=t2[:, :, 4:8], op=ADD)
            t4 = tp.tile([128, H, 2], BF16, tag="t4")
            nc.vector.tensor_tensor(out=t4[:], in0=t3[:, :, 0:2],
                                    in1=t3[:, :, 2:4], op=ADD)
            qks = tp.tile([128, H], BF16, tag="qks")
            nc.vector.tensor_tensor(out=qks[:], in0=t4[:, :, 0],
                                    in1=t4[:, :, 1], op=ADD)

            w_all = tp.tile([128, DEG, CARD, H], BF16, tag="wall")
            ctx_r = tp.tile([128, DEG, D], BF16, tag="ctxr")

            for d in range(DEG):
                # gather 7 co-member K|V rows (self pair in slot 0)
                CG = CARD - 1
                kvr = gp.tile([128, CG, 2 * D], BF16, tag="kvr")
                for cc in range(CG):
                    nc.gpsimd.indirect_dma_start(
                        out=kvr[:, cc, :], out_offset=None, in_=kv_tab[:],
                        in_offset=bass.IndirectOffsetOnAxis(
                            ap=pu_t[:, t, d * CARD + 1 + cc:d * CARD + 2 + cc],
                            axis=0))
                # K-score multiply on DVE (bf16 2x), slots 1..7
                prodk = sp_.tile([128, CG, D], BF16, tag="prodk")
                nc.vector.tensor_tensor(
                    out=prodk[:], in0=kvr[:, :, 0:D],
                    in1=q_t[:].unsqueeze(1).to_broadcast([128, CG, D]),
                    op=MUL)
                # score tree over e: D, D, P, P, P
                kv4 = prodk[:].rearrange("p c (h e) -> p c h e", h=H)
                s1 = sp_.tile([128, CG, H, 16], BF16, tag="s1")
                nc.vector.tensor_tensor(out=s1[:], in0=kv4[:, :, :, 0:16],
                                        in1=kv4[:, :, :, 16:32], op=ADD)
                s2 = sp_.tile([128, CG, H, 8], BF16, tag="s2")
                eng_s2 = nc.vector if d % 2 == 0 else nc.gpsimd
                eng_s2.tensor_tensor(out=s2[:], in0=s1[:, :, :, 0:8],
                                     in1=s1[:, :, :, 8:16], op=ADD)
                s3 = sp_.tile([128, CG, H, 4], BF16, tag="s3")
                eng_s3 = nc.vector if d % 2 == 0 else nc.gpsimd
                eng_s3.tensor_tensor(out=s3[:], in0=s2[:, :, :, 0:4],
                                     in1=s2[:, :, :, 4:8], op=ADD)
                s4 = sp_.tile([128, CG, H, 2], BF16, tag="s4")
                nc.gpsimd.tensor_tensor(out=s4[:], in0=s3[:, :, :, 0:2],
                                        in1=s3[:, :, :, 2:4], op=ADD)
                s5 = sp_.tile([128, CG, H], BF16, tag="s5")
                nc.gpsimd.tensor_tensor(out=s5[:], in0=s4[:, :, :, 0],
                                        in1=s4[:, :, :, 1], op=ADD)
                # + qek: slot 0 = self (qks), slots 1..7 from the tree
                sadd = sp_.tile([128, CARD, H], BF16, tag="sadd")
                nc.gpsimd.tensor_tensor(
                    out=sadd[:, 1:CARD, :], in0=s5[:],
                    in1=qek[:, d, :].unsqueeze(1).to_broadcast([128, CG, H]),
                    op=ADD)
                nc.gpsimd.tensor_tensor(out=sadd[:, 0, :], in0=qks[:],
                                        in1=qek[:, d, :], op=ADD)
                # w = exp(s) on Act -> w_all[:, d]
                nc.scalar.activation(out=w_all[:, d, :, :], in_=sadd[:],
                                     func=mybir.ActivationFunctionType.Exp)
                # wv[p, c, (e h)] = V * w  (w bcast over e; DVE bf16 2x)
                wv = sp_.tile([128, CARD, D], BF16, tag="wv")
                nc.vector.tensor_tensor(
                    out=wv[:, 1:CARD, :].rearrange("p c (e h) -> p c e h", h=H),
                    in0=kvr[:, :, D:2 * D].rearrange("p c (e h) -> p c e h",
                                                     h=H),
                    in1=w_all[:, d, 1:CARD, :].unsqueeze(2).to_broadcast(
                        [128, CG, DH, H]),
                    op=MUL)
                nc.vector.tensor_tensor(
                    out=wv[:, 0, :].rearrange("p (e h) -> p e h", h=H),
                    in0=skv[:, D:2 * D].rearrange("p (e h) -> p e h", h=H),
                    in1=w_all[:, d, 0, :].unsqueeze(1).to_broadcast(
                        [128, DH, H]),
                    op=MUL)
                # ctx tree over c -> ctx_r[:, d, :]: D, P, P
                c1 = sp_.tile([128, 4, D], BF16, tag="c1")
                nc.vector.tensor_tensor(out=c1[:], in0=wv[:, 0:4, :],
                                        in1=wv[:, 4:8, :], op=ADD)
                c2 = sp_.tile([128, 2, D], BF16, tag="c2")
                nc.gpsimd.tensor_tensor(out=c2[:], in0=c1[:, 0:2, :],
                                        in1=c1[:, 2:4, :], op=ADD)
                nc.vector.tensor_tensor(out=ctx_r[:, d, :], in0=c2[:, 0, :],
                                        in1=c2[:, 1, :], op=ADD)

            # ---- per-tile combine ----
            # z[p, h] = sum over (d, c) of w_all
            zsum = tp.tile([128, H], F32, tag="zsum")
            nc.vector.tensor_reduce(
                out=zsum[:], in_=w_all[:].rearrange("p d c h -> p (d c) h")
                .transpose([0, 2, 1]), axis=AX, op=ADD)
            zrec = tp.tile([128, H], F32, tag="zrec")
            nc.vector.reciprocal(out=zrec[:], in_=zsum[:])
            # ctx combine over d
            x1 = tp.tile([128, 4, D], BF16, tag="x1")
            nc.vector.tensor_tensor(out=x1[:], in0=ctx_r[:, 0:4, :],
                                    in1=ctx_r[:, 4:8, :], op=ADD)
            x2 = tp.tile([128, 2, D], BF16, tag="x2")
            nc.vector.tensor_tensor(out=x2[:], in0=x1[:, 0:2, :],
                                    in1=x1[:, 2:4, :], op=ADD)
            ctx = tp.tile([128, D], F32, tag="ctx")
            nc.vector.tensor_tensor(out=ctx[:], in0=x2[:, 0, :],
                                    in1=x2[:, 1, :], op=ADD)
            # normalize: ctxn[p, (e h)] = ctx * zrec[p, h] (bcast over e)
            ctxn = tp.tile([128, D], F32, tag="ctxn")
            nc.vector.tensor_tensor(
                out=ctxn[:].rearrange("p (e h) -> p e h", h=H),
                in0=ctx[:].rearrange("p (e h) -> p e h", h=H),
                in1=zrec[:].unsqueeze(1).to_broadcast([128, DH, H]), op=MUL)

            # out-proj: transpose ctxn, PE matmul (owT rows (e,h)-permuted)
            ctxT = tp.tile([128, 2, 128], BF16, tag="ctxT")
            for ch in range(2):
                ptr = psC.tile([128, 128], F32, space="PSUM", tag="ptr")
                nc.tensor.transpose(out=ptr[:], in_=ctxn[:, bass.ts(ch, 128)],
                                    identity=ident[:])
                nc.scalar.copy(out=ctxT[:, ch, :], in_=ptr[:])
            po = psB.tile([128, D], F32, space="PSUM", tag="p256")
            nc.tensor.matmul(out=po[:], lhsT=ctxT[:, 0, :], rhs=owT_t[:, 0, :],
                             start=True, stop=False)
            nc.tensor.matmul(out=po[:], lhsT=ctxT[:, 1, :], rhs=owT_t[:, 1, :],
                             start=False, stop=True)
            ob = tp.tile([128, D], F32, tag="ob")
            nc.vector.tensor_tensor(out=ob[:], in0=po[:], in1=bo_t[:], op=ADD)
            o_sb = tp.tile([128, D], F32, tag="osb")
            nc.scalar.activation(out=o_sb[:], in_=ob[:],
                                 func=mybir.ActivationFunctionType.Relu)
            nc.sync.dma_start(out=out[bass.ts(t, 128), :], in_=o_sb[:])

    return nc


# ---------------------------------------------------------------------------
def host_prep(x, incidence, edge_attr, W_lin, W_edge,
              in_proj_w, in_proj_b, out_proj_w, out_proj_b):
    x = np.asarray(x, np.float32)
    inc = np.asarray(incidence, np.float32)
    ea = np.asarray(edge_attr, np.float32)
    W_lin = np.asarray(W_lin, np.float32)
    W_edge = np.asarray(W_edge, np.float32)
    in_proj_w = np.asarray(in_proj_w, np.float32)
    in_proj_b = np.asarray(in_proj_b, np.float32)
    out_proj_w = np.asarray(out_proj_w, np.float32)
    out_proj_b = np.asarray(out_proj_b, np.float32)

    # index lists from incidence (order within a node's pair set is irrelevant:
    # attention is permutation-invariant over the L pairs)
    eon = np.nonzero(inc.T)[1].reshape(N, DEG).astype(np.int32)   # edge_of_node
    noe = np.nonzero(inc)[1].reshape(E, CARD).astype(np.int32)    # node_of_edge
    pair_u = noe[eon].astype(np.int32)                            # [N, DEG, CARD]
    # move the self pair to slot 0 of every round (order within a round's
    # pair set is irrelevant to attention)
    nidx = np.arange(N)[:, None]
    selfpos = np.argmax(pair_u == nidx[:, :, None].repeat(DEG, 1)
                        .reshape(N, DEG, 1), axis=2)
    for d_ in range(DEG):
        sp_ = selfpos[:, d_]
        row = pair_u[nidx[:, 0], d_]
        row[nidx[:, 0], sp_] = row[:, 0]
        row[:, 0] = np.arange(N)
        pair_u[:, d_, :] = row
    assert (pair_u[:, :, 0] == nidx).all()
    pair_u = pair_u.reshape(N, L)
    pair_e = eon

    Wq, Wk, Wv = in_proj_w[0:D], in_proj_w[D:2 * D], in_proj_w[2 * D:3 * D]
    bq, bk, bv = in_proj_b[0:D], in_proj_b[D:2 * D], in_proj_b[2 * D:3 * D]
    scale = 1.0 / np.sqrt(np.float32(DH))

    # V output columns permuted to (e, h)-major; owT rows likewise
    perm = (np.arange(D).reshape(H, DH).T.reshape(-1))  # perm[e*H+h] = h*DH+e
    wkc = (W_lin @ Wk.T).astype(np.float32)
    wvc = (W_lin @ Wv.T)[:, perm].astype(np.float32)
    wqc = (W_lin @ Wq.T * scale).astype(np.float32)
    wek = (W_edge @ Wk.T).astype(np.float32)
    owT = out_proj_w.T.copy().astype(np.float32)[perm, :]

    import ml_dtypes
    bf = ml_dtypes.bfloat16
    rep = dict(
        xT=np.ascontiguousarray(x.T).astype(bf),
        eaT=np.ascontiguousarray(ea.T).astype(bf),
        wkc=wkc.astype(bf), wvc=wvc.astype(bf), wqc=wqc.astype(bf),
        wek=wek.astype(bf), owT=owT.astype(bf),
        bkv_b=np.broadcast_to(np.concatenate([bk, np.zeros(D, np.float32)]),
                              (128, 2 * D)).copy(),
        bq_b=np.broadcast_to(bq * scale, (128, D)).copy(),
        bo_b=np.broadcast_to(out_proj_b + bv @ out_proj_w.T,
                             (128, D)).copy(),
    )
    per_core = []
    for c in range(NCORES):
        sl = slice(c * NSH, (c + 1) * NSH)
        m = dict(rep)
        m["xq_own"] = np.ascontiguousarray(x.T[:, sl]).astype(bf)
        m["pu"] = pair_u[sl]
        m["pe"] = pair_e[sl]
        m["self_idx"] = np.ascontiguousarray(
            np.arange(c * NSH, (c + 1) * NSH, dtype=np.int32)
            .reshape(NT, 128).T)
        per_core.append(m)
    return per_core


_CACHE = {}


def kernel(x, incidence, edge_attr, W_lin, W_edge,
           in_proj_w, in_proj_b, out_proj_w, out_proj_b, deg, card):
    assert int(deg) == DEG and int(card) == CARD
    in_maps = host_prep(x, incidence, edge_attr, W_lin, W_edge,
                        in_proj_w, in_proj_b, out_proj_w, out_proj_b)
    if "nc" not in _CACHE:
        _CACHE["nc"] = build_nc()
    from concourse.bass_utils import run_bass_kernel_spmd
    res = run_bass_kernel_spmd(_CACHE["nc"], in_maps, list(range(NCORES)))
    return np.concatenate([res.results[c]["out"] for c in range(NCORES)], axis=0)


# revision 51
# speedup vs baseline: 1.0142x; 1.0142x over previous
"""Trainium2 Bass kernel for CustomHyperSemanticMessagePassing.

Hypergraph multi-head attention message passing, N=4096 nodes, E=4096 edges,
DEG=CARD=8, D=256, H=8 heads. Sharding: data-parallel over nodes (512/core).

Final design (engine-balanced, bf16 tables, HW-validated gather form):
  - K|V table [N, 512] bf16 (K half (h,e) layout with bk folded; V half in
    (e,h) layout), EK table [E, 256] bf16, built on PE + converted via
    DVE/Act, stored to DRAM-pool tiles (dep-tracked vs the gathers).
  - Gathers use ONLY the single-index-per-partition indirect DMA form
    (multi-index offset APs are mis-ordered by real HW descriptor gen).
  - Self-pair trick: host reorders each round's pair set so slot 0 is the
    node itself; only 7 rows gathered per round, own row fetched once/tile
    and its q.K score reused across rounds.
  - Engine balance: Pool = gathers + small tree levels (s4,s5,sadd,c2);
    DVE = score/value multiplies (bf16 2x mode) + big tree levels +
    combines; Act = exp, phase-T V/EK conversion copies, PSUM copies,
    relu; PE = table matmuls, transposes, out-proj.
  - bv is folded into the output bias host-side (sum of softmax weights
    is 1 per head), so the V table needs no bias add.
  Softmax without max-subtraction (scores are O(1) bounded); per-round
  partial exp sums / weighted sums combine by plain addition.

"""
import numpy as np

import bass_rust
import orjson
import concourse.bass as bass
import concourse.tile as tile
import concourse.bass_utils as bass_utils
import concourse.bass2jax as bass2jax
from concourse import mybir
from concourse.masks import make_identity

F32 = mybir.dt.float32
BF16 = mybir.dt.bfloat16
FP8 = mybir.dt.float8e4
I32 = mybir.dt.int32

N, E, D, EDGE_DIM = 4096, 4096, 256, 64
H, DH, DEG, CARD = 8, 32, 8, 8
L = DEG * CARD
NCORES = 8
NSH = N // NCORES          # nodes per core
NT = NSH // 128            # 128-node tiles per core
NM = N // 128              # table tiles


# ---------------------------------------------------------------------------
# walrus workaround: this build accepts only one sync-wait per instruction;
# split extras into injected single-wait NoOps at the BIR-JSON level.
_ORIG_COMPILE = bass_utils.compile_bir_kernel
_ctr = [0]


def _split_multiwaits(bir_json: bytes) -> bytes:
    bir = orjson.loads(bir_json)
    changed = False
    for f in bir.get("functions", []):
        for blk in f.get("blocks", []):
            out = []
            for ins in blk.get("instructions", []):
                si = ins.get("sync_info")
                waits = (si or {}).get("on_wait") or []
                if len(waits) > 1 and ins.get("engine") not in (None, "Unassigned"):
                    changed = True
                    for w in waits[:-1]:
                        _ctr[0] += 1
                        out.append({
                            "debug": ins.get("debug"),
                            "engine": ins["engine"],
                            "ins": [], "outs": [],
                            "name": f"WSPLIT-{_ctr[0]}",
                            "opcode": "NoOp",
                            "sync_info": {"on_wait": [w], "on_update": []},
                        })
                    si["on_wait"] = waits[-1:]
                out.append(ins)
            blk["instructions"] = out
    return orjson.dumps(bir) if changed else bir_json


def _patched_compile(bir_json, tmpdir, neff_name="file.neff"):
    return _ORIG_COMPILE(_split_multiwaits(bytes(bir_json)), tmpdir,
                         neff_name=neff_name)


def _install_patch():
    bass_utils.compile_bir_kernel = _patched_compile
    bass2jax.compile_bir_kernel = _patched_compile


_install_patch()


# ---------------------------------------------------------------------------
def build_nc():
    nc = bass.Bass(num_devices=NCORES)
    # replicated inputs
    xT = nc.declare_dram_parameter("xT", [D, N], BF16, isOutput=False)
    eaT = nc.declare_dram_parameter("eaT", [EDGE_DIM, E], BF16, isOutput=False)
    wkc = nc.declare_dram_parameter("wkc", [D, D], BF16, isOutput=False)
    wvc = nc.declare_dram_parameter("wvc", [D, D], BF16, isOutput=False)
    wqc = nc.declare_dram_parameter("wqc", [D, D], BF16, isOutput=False)
    wek = nc.declare_dram_parameter("wek", [EDGE_DIM, D], BF16, isOutput=False)
    owT = nc.declare_dram_parameter("owT", [D, D], BF16, isOutput=False)
    bkv_b = nc.declare_dram_parameter("bkv_b", [128, 2 * D], F32, isOutput=False)
    bq_b = nc.declare_dram_parameter("bq_b", [128, D], F32, isOutput=False)
    bo_b = nc.declare_dram_parameter("bo_b", [128, D], F32, isOutput=False)
    # per-core inputs
    xq_own = nc.declare_dram_parameter("xq_own", [D, NSH], BF16, isOutput=False)
    pu = nc.declare_dram_parameter("pu", [NSH, L], I32, isOutput=False)
    pe = nc.declare_dram_parameter("pe", [NSH, DEG], I32, isOutput=False)
    self_idx = nc.declare_dram_parameter("self_idx", [128, NT], I32, isOutput=False)
    # output
    out = nc.declare_dram_parameter("out", [NSH, D], F32, isOutput=True)

    AX = mybir.AxisListType.X
    ADD = mybir.AluOpType.add
    MUL = mybir.AluOpType.mult

    with tile.TileContext(nc) as tc, \
         tc.tile_pool(name="wpool", bufs=1) as wp, \
         tc.tile_pool(name="stage", bufs=2) as stg, \
         tc.tile_pool(name="qpool", bufs=NT + 1) as qp, \
         tc.tile_pool(name="gpool", bufs=4) as gp, \
         tc.tile_pool(name="spool", bufs=4) as sp_, \
         tc.tile_pool(name="tpool", bufs=2) as tp, \
         tc.tile_pool(name="psA", bufs=2, space="PSUM") as psA, \
         tc.tile_pool(name="psB", bufs=2, space="PSUM") as psB, \
         tc.tile_pool(name="psC", bufs=2, space="PSUM") as psC, \
         tc.tile_pool(name="dpool", bufs=1, space="DRAM") as dpl:

        kv_tab = dpl.tile([N, 2 * D], BF16)
        ek_tab = dpl.tile([E, D], BF16)

        # ---- resident weights / biases / x / ea ----
        wk_t = wp.tile([128, 2, D], BF16)
        nc.sync.dma_start(out=wk_t[:], in_=wkc[:].rearrange("(c k) o -> k c o", c=2))
        wv_t = wp.tile([128, 2, D], BF16)
        nc.sync.dma_start(out=wv_t[:], in_=wvc[:].rearrange("(c k) o -> k c o", c=2))
        wq_t = wp.tile([128, 2, D], BF16)
        nc.sync.dma_start(out=wq_t[:], in_=wqc[:].rearrange("(c k) o -> k c o", c=2))
        wek_t = wp.tile([EDGE_DIM, D], BF16)
        nc.sync.dma_start(out=wek_t[:], in_=wek[:])
        owT_t = wp.tile([128, 2, D], BF16)
        nc.sync.dma_start(out=owT_t[:], in_=owT[:].rearrange("(c k) o -> k c o", c=2))
        bkv_t = wp.tile([128, 2 * D], F32)
        nc.scalar.dma_start(out=bkv_t[:], in_=bkv_b[:])
        bq_t = wp.tile([128, D], F32)
        nc.scalar.dma_start(out=bq_t[:], in_=bq_b[:])
        bo_t = wp.tile([128, D], F32)
        nc.scalar.dma_start(out=bo_t[:], in_=bo_b[:])
        ident = wp.tile([128, 128], F32)
        make_identity(nc, ident[:])

        # x resident: [k=128, chalf=2, m=32, n=128]; split load by chalf
        xt = wp.tile([128, 2, NM, 128], BF16)
        for ci, eng in ((0, nc.sync), (1, nc.scalar)):
            eng.dma_start(
                out=xt[:, ci, :, :],
                in_=xT[bass.ts(ci, 128), :].rearrange("k (m n) -> k m n", m=NM))
        # ea resident: [64, m=32, n=128]
        eat = wp.tile([EDGE_DIM, NM, 128], BF16)
        nc.sync.dma_start(
            out=eat[:], in_=eaT[:].rearrange("e (m n) -> e m n", m=NM))
        # own q source
        xqo = wp.tile([128, 2, NT, 128], BF16)
        nc.scalar.dma_start(
            out=xqo[:],
            in_=xq_own[:].rearrange("(c k) (t n) -> k c t n", c=2, t=NT))
        # indices
        pu_t = wp.tile([128, NT, L], I32)
        nc.sync.dma_start(
            out=pu_t[:], in_=pu[:].rearrange("(t p) l -> p t l", t=NT))
        pe_t = wp.tile([128, NT, DEG], I32)
        nc.sync.dma_start(
            out=pe_t[:], in_=pe[:].rearrange("(t p) l -> p t l", t=NT))
        sidx_t = wp.tile([128, NT], I32)
        nc.sync.dma_start(out=sidx_t[:], in_=self_idx[:])

        # ---- phase T: KV table ----
        for mg in range(NM // 4):

            k_st = stg.tile([128, 4, D], BF16, tag="kst")
            v_st = stg.tile([128, 4, D], BF16, tag="vst")
            for mi in range(0, 4, 2):
                pk2 = psA.tile([128, 2 * D], F32, space="PSUM", tag="pk2")
                pv2 = psA.tile([128, 2 * D], F32, space="PSUM", tag="pv2")
                for j in range(2):
                    m = mg * 4 + mi + j
                    nc.tensor.matmul(out=pk2[:, j * D:(j + 1) * D],
                                     lhsT=xt[:, 0, m, :], rhs=wk_t[:, 0, :],
                                     start=True, stop=False)
                    nc.tensor.matmul(out=pk2[:, j * D:(j + 1) * D],
                                     lhsT=xt[:, 1, m, :], rhs=wk_t[:, 1, :],
                                     start=False, stop=True)
                    nc.tensor.matmul(out=pv2[:, j * D:(j + 1) * D],
                                     lhsT=xt[:, 0, m, :], rhs=wv_t[:, 0, :],
                                     start=True, stop=False)
                    nc.tensor.matmul(out=pv2[:, j * D:(j + 1) * D],
                                     lhsT=xt[:, 1, m, :], rhs=wv_t[:, 1, :],
                                     start=False, stop=True)
                nc.vector.tensor_tensor(
                    out=k_st[:, mi:mi + 2, :],
                    in0=pk2[:].rearrange("p (j d) -> p j d", j=2),
                    in1=bkv_t[:, 0:D].unsqueeze(1).to_broadcast([128, 2, D]),
                    op=ADD)
                if mi % 4 == 0:
                    nc.scalar.copy(
                        out=v_st[:, mi:mi + 2, :],
                        in_=pv2[:].rearrange("p (j d) -> p j d", j=2))
                else:
                    nc.vector.tensor_scalar(
                        out=v_st[:, mi:mi + 2, :],
                        in0=pv2[:].rearrange("p (j d) -> p j d", j=2),
                        scalar1=0.0, scalar2=None, op0=ADD)

            kv_rows = kv_tab[bass.ts(mg, 512), :].rearrange(
                "(s p) d -> p s d", s=4)
            eng = (nc.sync, nc.scalar)[mg % 2]
            eng.dma_start(out=kv_rows[:, :, 0:D], in_=k_st[:])
            eng = (nc.scalar, nc.sync)[mg % 2]
            eng.dma_start(out=kv_rows[:, :, D:2 * D], in_=v_st[:])

        # ---- phase T: EK table ----
        for mg in range(NM // 4):
            ek_st = stg.tile([128, 4, D], BF16, tag="ekst")
            for mi in range(0, 4, 2):
                pe2 = psA.tile([128, 2 * D], F32, space="PSUM", tag="pk2")
                for j in range(2):
                    m = mg * 4 + mi + j
                    nc.tensor.matmul(out=pe2[:, j * D:(j + 1) * D],
                                     lhsT=eat[:, m, :], rhs=wek_t[:],
                                     start=True, stop=True)
                nc.scalar.copy(
                    out=ek_st[:, mi:mi + 2, :],
                    in_=pe2[:].rearrange("p (j d) -> p j d", j=2))
            eng = (nc.sync, nc.scalar)[mg % 2]
            eng.dma_start(
                out=ek_tab[bass.ts(mg, 512), :].rearrange("(s p) d -> p s d", s=4),
                in_=ek_st[:])

        # ---- phase T: q for own nodes ----
        q_tiles = []
        for t in range(NT):
            pq = psB.tile([128, D], F32, space="PSUM", tag="p256")
            nc.tensor.matmul(out=pq[:], lhsT=xqo[:, 0, t, :], rhs=wq_t[:, 0, :],
                             start=True, stop=False)
            nc.tensor.matmul(out=pq[:], lhsT=xqo[:, 1, t, :], rhs=wq_t[:, 1, :],
                             start=False, stop=True)
            q_t = qp.tile([128, D], BF16, tag=f"q{t}")
            nc.vector.tensor_tensor(out=q_t[:], in0=pq[:], in1=bq_t[:], op=ADD)
            q_tiles.append(q_t)

        # ---- phase A ----
        for t in range(NT):
            q_t = q_tiles[t]
            # EK gathers (single-index per partition; HW-validated form)
            ekg = gp.tile([128, DEG, D], BF16, tag="ekg")
            for dg in range(DEG):
                nc.gpsimd.indirect_dma_start(
                    out=ekg[:, dg, :], out_offset=None, in_=ek_tab[:],
                    in_offset=bass.IndirectOffsetOnAxis(
                        ap=pe_t[:, t, dg:dg + 1], axis=0))
            # prode[p, d, (h e)] = EK * q  (Pool reads fp8)
            prode = tp.tile([128, DEG, D], BF16, tag="prode")
            nc.vector.tensor_tensor(
                out=prode[:], in0=ekg[:],
                in1=q_t[:].unsqueeze(1).to_broadcast([128, DEG, D]), op=MUL)
            # qek tree over e: [128, d, h, 32] -> [128, d, h]
            pv = prode[:].rearrange("p d (h e) -> p d h e", h=H)
            qk1 = tp.tile([128, DEG, H, 16], BF16, tag="qk1")
            nc.vector.tensor_tensor(out=qk1[:], in0=pv[:, :, :, 0:16],
                                    in1=pv[:, :, :, 16:32], op=ADD)
            qk2 = tp.tile([128, DEG, H, 8], BF16, tag="qk2")
            nc.vector.tensor_tensor(out=qk2[:], in0=qk1[:, :, :, 0:8],
                                    in1=qk1[:, :, :, 8:16], op=ADD)
            qk3 = tp.tile([128, DEG, H, 4], BF16, tag="qk3")
            nc.vector.tensor_tensor(out=qk3[:], in0=qk2[:, :, :, 0:4],
                                    in1=qk2[:, :, :, 4:8], op=ADD)
            qk4 = tp.tile([128, DEG, H, 2], BF16, tag="qk4")
            nc.vector.tensor_tensor(out=qk4[:], in0=qk3[:, :, :, 0:2],
                                    in1=qk3[:, :, :, 2:4], op=ADD)
            qek = tp.tile([128, DEG, H], BF16, tag="qek")
            nc.vector.tensor_tensor(out=qek[:], in0=qk4[:, :, :, 0],
                                    in1=qk4[:, :, :, 1], op=ADD)

            # own K|V row (self pair lives in slot 0 of every round)
            skv = tp.tile([128, 2 * D], BF16, tag="skv")
            nc.gpsimd.indirect_dma_start(
                out=skv[:], out_offset=None, in_=kv_tab[:],
                in_offset=bass.IndirectOffsetOnAxis(
                    ap=sidx_t[:, t:t + 1], axis=0))
            pks = tp.tile([128, D], BF16, tag="pks")
            nc.vector.tensor_tensor(out=pks[:], in0=skv[:, 0:D], in1=q_t[:],
                                    op=MUL)
            pk4 = pks[:].rearrange("p (h e) -> p h e", h=H)
            t1 = tp.tile([128, H, 16], BF16, tag="t1")
            nc.vector.tensor_tensor(out=t1[:], in0=pk4[:, :, 0:16],
                                    in1=pk4[:, :, 16:32], op=ADD)
            t2 = tp.tile([128, H, 8], BF16, tag="t2")
            nc.vector.tensor_tensor(out=t2[:], in0=t1[:, :, 0:8],
                                    in1=t1[:, :, 8:16], op=ADD)
            t3 = tp.tile([128, H, 4], BF16, tag="t3")
            nc.vector.tensor_tensor(out=t3[:], in0=t2[:, :, 0:4],
                                    in1=t2[:, :, 4:8], op=ADD)
            t4 = tp.tile([128, H, 2], BF16, tag="t4")
            nc.vector.tensor_tensor(out=t4[:], in0=t3[:, :, 0:2],
                                    in1=t3[:, :, 2:4], op=ADD)
            qks = tp.tile([128, H], BF16, tag="qks")
            nc.vector.tensor_tensor(out=qks[:], in0=t4[:, :, 0],
                                    in1=t4[:, :, 1], op=ADD)

            w_all = tp.tile([128, DEG, CARD, H], BF16, tag="wall")
            ctx_r = tp.tile([128, DEG, D], BF16, tag="ctxr")

            for d in range(DEG):
                # gather 7 co-member K|V rows (self pair in slot 0)
                CG = CARD - 1
                kvr = gp.tile([128, CG, 2 * D], BF16, tag="kvr")
                for cc in range(CG):
                    nc.gpsimd.indirect_dma_start(
                        out=kvr[:, cc, :], out_offset=None, in_=kv_tab[:],
                        in_offset=bass.IndirectOffsetOnAxis(
                            ap=pu_t[:, t, d * CARD + 1 + cc:d * CARD + 2 + cc],
                            axis=0))
                # K-score multiply on DVE (bf16 2x), slots 1..7
                prodk = sp_.tile([128, CG, D], BF16, tag="prodk")
                nc.vector.tensor_tensor(
                    out=prodk[:], in0=kvr[:, :, 0:D],
                    in1=q_t[:].unsqueeze(1).to_broadcast([128, CG, D]),
                    op=MUL)
                # score tree over e: D, D, P, P, P
                kv4 = prodk[:].rearrange("p c (h e) -> p c h e", h=H)
                s1 = sp_.tile([128, CG, H, 16], BF16, tag="s1")
                nc.vector.tensor_tensor(out=s1[:], in0=kv4[:, :, :, 0:16],
                                        in1=kv4[:, :, :, 16:32], op=ADD)
                s2 = sp_.tile([128, CG, H, 8], BF16, tag="s2")
                nc.vector.tensor_tensor(out=s2[:], in0=s1[:, :, :, 0:8],
                                        in1=s1[:, :, :, 8:16], op=ADD)
                s3 = sp_.tile([128, CG, H, 4], BF16, tag="s3")
                nc.vector.tensor_tensor(out=s3[:], in0=s2[:, :, :, 0:4],
                                        in1=s2[:, :, :, 4:8], op=ADD)
                s4 = sp_.tile([128, CG, H, 2], BF16, tag="s4")
                nc.gpsimd.tensor_tensor(out=s4[:], in0=s3[:, :, :, 0:2],
                                        in1=s3[:, :, :, 2:4], op=ADD)
                s5 = sp_.tile([128, CG, H], BF16, tag="s5")
                nc.gpsimd.tensor_tensor(out=s5[:], in0=s4[:, :, :, 0],
                                        in1=s4[:, :, :, 1], op=ADD)
                # + qek: slot 0 = self (qks), slots 1..7 from the tree
                sadd = sp_.tile([128, CARD, H], BF16, tag="sadd")
                nc.gpsimd.tensor_tensor(
                    out=sadd[:, 1:CARD, :], in0=s5[:],
                    in1=qek[:, d, :].unsqueeze(1).to_broadcast([128, CG, H]),
                    op=ADD)
                nc.gpsimd.tensor_tensor(out=sadd[:, 0, :], in0=qks[:],
                                        in1=qek[:, d, :], op=ADD)
                # w = exp(s) on Act -> w_all[:, d]
                nc.scalar.activation(out=w_all[:, d, :, :], in_=sadd[:],
                                     func=mybir.ActivationFunctionType.Exp)
                # wv[p, c, (e h)] = V * w  (w bcast over e; DVE bf16 2x)
                wv = sp_.tile([128, CARD, D], BF16, tag="wv")
                nc.vector.tensor_tensor(
                    out=wv[:, 1:CARD, :].rearrange("p c (e h) -> p c e h", h=H),
                    in0=kvr[:, :, D:2 * D].rearrange("p c (e h) -> p c e h",
                                                     h=H),
                    in1=w_all[:, d, 1:CARD, :].unsqueeze(2).to_broadcast(
                        [128, CG, DH, H]),
                    op=MUL)
                nc.vector.tensor_tensor(
                    out=wv[:, 0, :].rearrange("p (e h) -> p e h", h=H),
                    in0=skv[:, D:2 * D].rearrange("p (e h) -> p e h", h=H),
                    in1=w_all[:, d, 0, :].unsqueeze(1).to_broadcast(
                        [128, DH, H]),
                    op=MUL)
                # ctx tree over c -> ctx_r[:, d, :]: D, P, P
                c1 = sp_.tile([128, 4, D], BF16, tag="c1")
                nc.vector.tensor_tensor(out=c1[:], in0=wv[:, 0:4, :],
                                        in1=wv[:, 4:8, :], op=ADD)
                c2 = sp_.tile([128, 2, D], BF16, tag="c2")
                nc.gpsimd.tensor_tensor(out=c2[:], in0=c1[:, 0:2, :],
                                        in1=c1[:, 2:4, :], op=ADD)
                nc.vector.tensor_tensor(out=ctx_r[:, d, :], in0=c2[:, 0, :],
                                        in1=c2[:, 1, :], op=ADD)

            # ---- per-tile combine ----
            # z[p, h] = sum over (d, c) of w_all
            zsum = tp.tile([128, H], F32, tag="zsum")
            nc.vector.tensor_reduce(
                out=zsum[:], in_=w_all[:].rearrange("p d c h -> p (d c) h")
                .transpose([0, 2, 1]), axis=AX, op=ADD)
            zrec = tp.tile([128, H], F32, tag="zrec")
            nc.vector.reciprocal(out=zrec[:], in_=zsum[:])
            # ctx combine over d
            x1 = tp.tile([128, 4, D], BF16, tag="x1")
            nc.vector.tensor_tensor(out=x1[:], in0=ctx_r[:, 0:4, :],
                                    in1=ctx_r[:, 4:8, :], op=ADD)
            x2 = tp.tile([128, 2, D], BF16, tag="x2")
            nc.vector.tensor_tensor(out=x2[:], in0=x1[:, 0:2, :],
                                    in1=x1[:, 2:4, :], op=ADD)
            ctx = tp.tile([128, D], F32, tag="ctx")
            nc.vector.tensor_tensor(out=ctx[:], in0=x2[:, 0, :],
                                    in1=x2[:, 1, :], op=ADD)
            # normalize: ctxn[p, (e h)] = ctx * zrec[p, h] (bcast over e)
            ctxn = tp.tile([128, D], F32, tag="ctxn")
            nc.vector.tensor_tensor(
                out=ctxn[:].rearrange("p (e h) -> p e h", h=H),
                in0=ctx[:].rearrange("p (e h) -> p e h", h=H),
                in1=zrec[:].unsqueeze(1).to_broadcast([128, DH, H]), op=MUL)

            # out-proj: transpose ctxn, PE matmul (owT rows (e,h)-permuted)
            ctxT = tp.tile([128, 2, 128], BF16, tag="ctxT")
            for ch in range(2):
                ptr = psC.tile([128, 128], F32, space="PSUM", tag="ptr")
                nc.tensor.transpose(out=ptr[:], in_=ctxn[:, bass.ts(ch, 128)],
                                    identity=ident[:])
                nc.scalar.copy(out=ctxT[:, ch, :], in_=ptr[:])
            po = psB.tile([128, D], F32, space="PSUM", tag="p256")
            nc.tensor.matmul(out=po[:], lhsT=ctxT[:, 0, :], rhs=owT_t[:, 0, :],
                             start=True, stop=False)
            nc.tensor.matmul(out=po[:], lhsT=ctxT[:, 1, :], rhs=owT_t[:, 1, :],
                             start=False, stop=True)
            ob = tp.tile([128, D], F32, tag="ob")
            nc.vector.tensor_tensor(out=ob[:], in0=po[:], in1=bo_t[:], op=ADD)
            o_sb = tp.tile([128, D], F32, tag="osb")
            nc.scalar.activation(out=o_sb[:], in_=ob[:],
                                 func=mybir.ActivationFunctionType.Relu)
            nc.sync.dma_start(out=out[bass.ts(t, 128), :], in_=o_sb[:])

    return nc


# ---------------------------------------------------------------------------
def host_prep(x, incidence, edge_attr, W_lin, W_edge,
              in_proj_w, in_proj_b, out_proj_w, out_proj_b):
    x = np.asarray(x, np.float32)
    inc = np.asarray(incidence, np.float32)
    ea = np.asarray(edge_attr, np.float32)
    W_lin = np.asarray(W_lin, np.float32)
    W_edge = np.asarray(W_edge, np.float32)
    in_proj_w = np.asarray(in_proj_w, np.float32)
    in_proj_b = np.asarray(in_proj_b, np.float32)
    out_proj_w = np.asarray(out_proj_w, np.float32)
    out_proj_b = np.asarray(out_proj_b, np.float32)

    # index lists from incidence (order within a node's pair set is irrelevant:
    # attention is permutation-invariant over the L pairs)
    eon = np.nonzero(inc.T)[1].reshape(N, DEG).astype(np.int32)   # edge_of_node
    noe = np.nonzero(inc)[1].reshape(E, CARD).astype(np.int32)    # node_of_edge
    pair_u = noe[eon].astype(np.int32)                            # [N, DEG, CARD]
    # move the self pair to slot 0 of every round (order within a round's
    # pair set is irrelevant to attention)
    nidx = np.arange(N)[:, None]
    selfpos = np.argmax(pair_u == nidx[:, :, None].repeat(DEG, 1)
                        .reshape(N, DEG, 1), axis=2)
    for d_ in range(DEG):
        sp_ = selfpos[:, d_]
        row = pair_u[nidx[:, 0], d_]
        row[nidx[:, 0], sp_] = row[:, 0]
        row[:, 0] = np.arange(N)
        pair_u[:, d_, :] = row
    assert (pair_u[:, :, 0] == nidx).all()
    pair_u = pair_u.reshape(N, L)
    pair_e = eon

    Wq, Wk, Wv = in_proj_w[0:D], in_proj_w[D:2 * D], in_proj_w[2 * D:3 * D]
    bq, bk, bv = in_proj_b[0:D], in_proj_b[D:2 * D], in_proj_b[2 * D:3 * D]
    scale = 1.0 / np.sqrt(np.float32(DH))

    # V output columns permuted to (e, h)-major; owT rows likewise
    perm = (np.arange(D).reshape(H, DH).T.reshape(-1))  # perm[e*H+h] = h*DH+e
    wkc = (W_lin @ Wk.T).astype(np.float32)
    wvc = (W_lin @ Wv.T)[:, perm].astype(np.float32)
    wqc = (W_lin @ Wq.T * scale).astype(np.float32)
    wek = (W_edge @ Wk.T).astype(np.float32)
    owT = out_proj_w.T.copy().astype(np.float32)[perm, :]

    import ml_dtypes
    bf = ml_dtypes.bfloat16
    rep = dict(
        xT=np.ascontiguousarray(x.T).astype(bf),
        eaT=np.ascontiguousarray(ea.T).astype(bf),
        wkc=wkc.astype(bf), wvc=wvc.astype(bf), wqc=wqc.astype(bf),
        wek=wek.astype(bf), owT=owT.astype(bf),
        bkv_b=np.broadcast_to(np.concatenate([bk, np.zeros(D, np.float32)]),
                              (128, 2 * D)).copy(),
        bq_b=np.broadcast_to(bq * scale, (128, D)).copy(),
        bo_b=np.broadcast_to(out_proj_b + bv @ out_proj_w.T,
                             (128, D)).copy(),
    )
    per_core = []
    for c in range(NCORES):
        sl = slice(c * NSH, (c + 1) * NSH)
        m = dict(rep)
        m["xq_own"] = np.ascontiguousarray(x.T[:, sl]).astype(bf)
        m["pu"] = pair_u[sl]
        m["pe"] = pair_e[sl]
        m["self_idx"] = np.ascontiguousarray(
            np.arange(c * NSH, (c + 1) * NSH, dtype=np.int32)
            .reshape(NT, 128).T)
        per_core.append(m)
    return per_core


_CACHE = {}


def kernel(x, incidence, edge_attr, W_lin, W_edge,
           in_proj_w, in_proj_b, out_proj_w, out_proj_b, deg, card):
    assert int(deg) == DEG and int(card) == CARD
    in_maps = host_prep(x, incidence, edge_attr, W_lin, W_edge,
                        in_proj_w, in_proj_b, out_proj_w, out_proj_b)
    if "nc" not in _CACHE:
        _CACHE["nc"] = build_nc()
    from concourse.bass_utils import run_bass_kernel_spmd
    res = run_bass_kernel_spmd(_CACHE["nc"], in_maps, list(range(NCORES)))
    return np.concatenate([res.results[c]["out"] for c in range(NCORES)], axis=0)
